# revision 44
# baseline (speedup 1.0000x reference)
"""Trainium2 Bass kernel for nn_Attention_13039520711118 (attention pooling).

reference:
    h = hidden[:, -1, :]
    m = enc @ M_w[:, :E].T + h @ M_w[:, E:].T + M_b        # (B, S, H)
    scores = tanh(m) @ V_w[0] + V_b                        # (B, S)
    scores = where(mask, -1e9, scores)
    weights = softmax(scores, axis=1)[:, None, :]          # (B, 1, S)
    weighted = weights @ enc                               # (B, 1, E)
    return weighted, weights

Sharding: data-parallel over batch B=16 across 8 cores (2 batches/core);
M_w / M_b / V_w replicated.

Per-core pipeline, single pass over encoded (all shapes hardcoded):
  encoded is declared float32r in DRAM (same bits as f32) so plain DMAs feed
  both consumers with no casting DMA (SWDGE dtype-cast DMAs measured ~20x
  slow).  Per 512-column s-chunk:
    PE-transpose the 4 [128,2048] f32r s-tiles into PSUM; ACT copies
    convert to bf16 encT tiles.  mT[h,s] = sum_e M_eT.T @ encT accumulated
    in PSUM f32 (bf16 matmuls); tanh(+per-h bias) on ACT -> bf16;
    scores = V.T @ tanh on PE (M=1 matmuls).  Chunk scores are masked and
    exp'd with a constant shift exp(s - 32) (|scores| <= ||V||_1 <= 32 so no
    overflow; softmax is shift-invariant so the shift and the dropped V_b
    both cancel).  The exp'd chunk transposes to a [128,4] f32r column
    vector and immediately accumulates weighted_partial = expT.T @ enc
    on PE (f32r, ~1e-4 rel err) while the f32r s-tiles are still in SBUF —
    no second read of encoded.
  Final per batch: expv = exp(scores - 32) with accum_out Z on ACT,
  weights = expv / Z, weighted = acc / Z.
  Bias = M_h @ h + M_b on DVE (f32 multiply + reduce against a partition-
  broadcast copy of h).  Masked entries round to exactly -1e9 in f32,
  matching the reference's fill, so masked weights are exactly 0 both ways.
"""
import sys

sys.path.insert(0, "/opt/trn_rl_repo")

from contextlib import ExitStack

import numpy as np

import concourse.bacc as bacc
import concourse.bass as bass
import concourse.mybir as mybir
import concourse.tile as tile
from concourse import masks
from concourse.bass_utils import run_bass_kernel_spmd

F32 = mybir.dt.float32
F32R = mybir.dt.float32r
BF16 = mybir.dt.bfloat16
U8 = mybir.dt.uint8
AF = mybir.ActivationFunctionType
ALU = mybir.AluOpType
AX = mybir.AxisListType

N_CORES = 8
B, S, E, H = 16, 2048, 2048, 1024
BPC = B // N_CORES          # batches per core
SC = 512                    # s-chunk (columns per mm1 matmul)
NSC = S // SC               # 4 s-chunks per batch
NET = E // 128              # 16 e-tiles
NHT = H // 128              # 8 h-tiles
NST = S // 128              # 16 s-tiles
HG = 2                      # h-tiles per psum group
NEG = -1e9
MSHIFT = -32.0              # exp shift; |scores| <= ||V||_1 <= sqrt(H) = 32

LAST_EXEC_NS = None         # set by test harness runs with trace=True


def _build():
    nc = bacc.Bacc("TRN2", target_bir_lowering=False, debug=False,
                   num_devices=N_CORES)

    enc_d = nc.dram_tensor("enc", [BPC, S, E], F32R, kind="ExternalInput")
    hid_d = nc.dram_tensor("hid", [BPC, H], F32, kind="ExternalInput")
    mask_d = nc.dram_tensor("mask", [BPC, S], U8, kind="ExternalInput")
    mw_d = nc.dram_tensor("mw", [H, E + H], F32, kind="ExternalInput")
    mbT_d = nc.dram_tensor("mbT", [128, NHT], F32, kind="ExternalInput")
    vT_d = nc.dram_tensor("vT", [128, NHT], F32, kind="ExternalInput")

    w_o = nc.dram_tensor("w_o", [BPC, S], F32, kind="ExternalOutput")
    ws_o = nc.dram_tensor("ws_o", [BPC, E], F32, kind="ExternalOutput")

    with tile.TileContext(nc) as tc, ExitStack() as ctx:
        const = ctx.enter_context(tc.tile_pool(name="const", bufs=1))
        meTl_p = ctx.enter_context(tc.tile_pool(name="meTl", bufs=NET))
        meTh_p = ctx.enter_context(tc.tile_pool(name="meTh", bufs=NET))
        nat_p = ctx.enter_context(tc.tile_pool(name="nat", bufs=8))
        mh_p = ctx.enter_context(tc.tile_pool(name="mh", bufs=2))
        e512_p = ctx.enter_context(tc.tile_pool(name="e512", bufs=22))
        tanh_p = ctx.enter_context(tc.tile_pool(name="tanh", bufs=6))
        vec_p = ctx.enter_context(tc.tile_pool(name="vec", bufs=5))
        cvec_p = ctx.enter_context(tc.tile_pool(name="cvec", bufs=2))
        small_p = ctx.enter_context(tc.tile_pool(name="small", bufs=2))
        acc_p = ctx.enter_context(tc.tile_pool(name="acc", bufs=4, space="PSUM"))
        wacc_p = ctx.enter_context(tc.tile_pool(name="wacc", bufs=2, space="PSUM"))
        aux_p = ctx.enter_context(tc.tile_pool(name="aux", bufs=2, space="PSUM"))

        # ---------------- constants ----------------
        ident_f32 = const.tile([128, 128], F32)
        masks.make_identity(nc, ident_f32[:])
        ident_r = const.tile([128, 128], F32R)
        nc.vector.tensor_copy(ident_r[:], ident_f32[:])
        one1 = const.tile([1, 1], F32)
        nc.gpsimd.memset(one1[:], 1.0)
        msh = const.tile([1, 1], F32)
        nc.gpsimd.memset(msh[:], MSHIFT)

        # PE warmup: ~11us of back-to-back identity matmuls while the first
        # DMAs stream in, so HAM reaches K=8/8 before real matmuls start.
        ident16 = const.tile([128, 128], BF16)
        nc.vector.tensor_copy(ident16[:], ident_f32[:])
        wps = aux_p.tile([128, 128], F32, tag="aux", name="warmps")
        for i in range(180):
            nc.tensor.matmul(wps[:], ident16[:], ident16[:],
                             start=(i == 0), stop=(i == 179))

        vT = const.tile([128, NHT], BF16)
        nc.gpsimd.dma_start(vT[:], vT_d[:, :])          # cast f32 -> bf16
        mbT = const.tile([128, NHT], F32)
        nc.sync.dma_start(mbT[:], mbT_d[:, :])

        mask_sb = []
        for b in range(BPC):
            t = const.tile([1, S], U8, name=f"mask{b}")
            nc.sync.dma_start(t[:], mask_d[b:b + 1, :])
            mask_sb.append(t)

        # h broadcast across partitions: [128, H] per batch (f32)
        hbc = []
        for b in range(BPC):
            t = const.tile([128, H], F32, name=f"hbc{b}")
            nc.sync.dma_start(t[:], bass.AP(hid_d, b * H, [[0, 128], [1, H]]))
            hbc.append(t)

        bias_sb = const.tile([128, NHT * BPC], F32)     # col = ht*BPC + b

        # ---------------- helpers ----------------
        def load_chunk(b, sc):
            nat4 = []
            for j in range(SC // 128):
                st = sc * (SC // 128) + j
                t = nat_p.tile([128, E], F32R, tag="nat", name=f"nat{b}_{st}")
                nc.gpsimd.dma_start(t[:], enc_d[b, st * 128:(st + 1) * 128, :])
                nat4.append(t)
            return nat4

        def transpose_chunk(b, sc, nat4):
            encT = []
            for et in range(NET):
                pt = aux_p.tile([128, SC], F32R, tag="aux",
                                name=f"tp{b}_{sc}_{et}")
                for j in range(SC // 128):
                    nc.tensor.transpose(
                        pt[:, j * 128:(j + 1) * 128],
                        nat4[j][:, et * 128:(et + 1) * 128], ident_r[:])
                t = e512_p.tile([128, SC], BF16, tag="e512",
                                name=f"encT{b}_{sc}_{et}")
                nc.scalar.copy(t[:], pt[:])
                encT.append(t)
            return encT

        def me_group(hh, dest_tiles):
            group = []
            for ht in range(hh, hh + 4):
                t = nat_p.tile([128, E], F32, tag="nat", name=f"menat{ht}")
                nc.sync.dma_start(t[:], mw_d[ht * 128:(ht + 1) * 128, 0:E])
                group.append(t)
            for et in range(NET):
                pt = aux_p.tile([128, 512], F32, tag="aux",
                                name=f"metp{hh}_{et}")
                for i in range(4):
                    nc.tensor.transpose(
                        pt[:, i * 128:(i + 1) * 128],
                        group[i][:, et * 128:(et + 1) * 128], ident_f32[:])
                nc.scalar.copy(dest_tiles[et][:], pt[:])

        def bias_setup():
            for ht in range(NHT):
                pps = {}
                for dh in range(2):                      # H = 2 x 512
                    mh_t = mh_p.tile([128, 512], F32, tag="mh",
                                     name=f"mh{ht}_{dh}")
                    nc.sync.dma_start(
                        mh_t[:], mw_d[ht * 128:(ht + 1) * 128,
                                      E + dh * 512:E + (dh + 1) * 512])
                    for b in range(BPC):
                        scr = small_p.tile([128, 512], F32, tag="bscr",
                                           name=f"scr{b}_{ht}_{dh}")
                        nc.vector.tensor_mul(
                            scr[:], mh_t[:],
                            hbc[b][:, dh * 512:(dh + 1) * 512])
                        pp = small_p.tile([128, 1], F32, tag=f"pp{b}_{dh}",
                                          name=f"pp{b}_{ht}_{dh}")
                        nc.vector.tensor_reduce(pp[:], scr[:], axis=AX.X,
                                                op=ALU.add)
                        pps[(b, dh)] = pp
                for b in range(BPC):
                    col = ht * BPC + b
                    ps = small_p.tile([128, 1], F32, tag="psum01",
                                      name=f"ps{b}_{ht}")
                    nc.vector.tensor_add(ps[:], pps[(b, 0)][:],
                                         pps[(b, 1)][:])
                    nc.vector.tensor_add(bias_sb[:, col:col + 1], ps[:],
                                         mbT[:, ht:ht + 1])

        def mm1_chunk(b, sc, encT):
            """matmuls + tanh + V-dot; returns the scores psum tile."""
            tanh_tiles = []
            for hg in range(NHT // HG):
                accs = [acc_p.tile([128, SC], F32, tag="acc",
                                   name=f"acc{b}_{sc}_{hg}_{hh}")
                        for hh in range(HG)]
                for et in range(NET):
                    for hh in range(HG):
                        ht = hg * HG + hh
                        src = meTl[et] if ht < 4 else meTh[et]
                        co = (ht % 4) * 128
                        nc.tensor.matmul(
                            accs[hh][:, :], src[:, co:co + 128],
                            encT[et][:, :],
                            start=(et == 0), stop=(et == NET - 1))
                for hh in range(HG):
                    ht = hg * HG + hh
                    tt = tanh_p.tile([128, SC], BF16, tag="tanh",
                                     name=f"tanh{b}_{sc}_{hg}_{hh}")
                    nc.scalar.activation(
                        tt[:], accs[hh][:], AF.Tanh,
                        bias=bias_sb[:, ht * BPC + b:ht * BPC + b + 1])
                    tanh_tiles.append(tt)
            sc_ps = aux_p.tile([1, SC], F32, tag="aux", name=f"scps{b}_{sc}")
            for ht in range(NHT):
                nc.tensor.matmul(sc_ps[:, :], vT[:, ht:ht + 1],
                                 tanh_tiles[ht][:, :],
                                 start=(ht == 0), stop=(ht == NHT - 1))
            return sc_ps

        def chunk_scores(b, sc, sc_ps, ssb):
            """mask + store raw masked scores, exp(s-32), transpose to f32r."""
            mnegc = cvec_p.tile([1, SC], F32, tag="cvec", name=f"mng{b}_{sc}")
            nc.vector.tensor_scalar_mul(mnegc[:],
                                        mask_sb[b][:, sc * SC:(sc + 1) * SC],
                                        NEG)
            nc.vector.tensor_add(ssb[:, sc * SC:(sc + 1) * SC], sc_ps[:],
                                 mnegc[:])
            expc = cvec_p.tile([1, SC], F32, tag="cvec", name=f"exc{b}_{sc}")
            nc.scalar.activation(expc[:], ssb[:, sc * SC:(sc + 1) * SC],
                                 AF.Exp, bias=msh[:, 0:1])
            ept = aux_p.tile([128, SC // 128], F32, tag="aux",
                             name=f"ept{b}_{sc}")
            for j in range(SC // 128):
                nc.tensor.transpose(ept[:, j:j + 1],
                                    expc[0:1, j * 128:(j + 1) * 128], one1[:])
            expT = small_p.tile([128, SC // 128], F32R, tag="expT",
                                name=f"expT{b}_{sc}")
            nc.vector.tensor_copy(expT[:], ept[:])
            return expT

        def weighted_partial(b, sc, nat4, expT, acc_sb):
            """acc_sb[0, :] += sum_j expT[:, j].T @ nat4[j]  (f32r on PE)."""
            for ec in range(4):
                wp = wacc_p.tile([1, 512], F32, tag="wacc",
                                 name=f"wp{b}_{sc}_{ec}")
                for j in range(SC // 128):
                    nc.tensor.matmul(
                        wp[:, :], expT[:, j:j + 1],
                        nat4[j][:, ec * 512:(ec + 1) * 512],
                        start=(j == 0), stop=(j == SC // 128 - 1))
                if sc == 0:
                    nc.vector.tensor_copy(
                        acc_sb[:, ec * 512:(ec + 1) * 512], wp[:])
                else:
                    nc.vector.tensor_add(
                        acc_sb[:, ec * 512:(ec + 1) * 512],
                        acc_sb[:, ec * 512:(ec + 1) * 512], wp[:])

        def finalize(b, ssb, acc_sb):
            expv = vec_p.tile([1, S], F32, tag="vec", name=f"expv{b}")
            zs = small_p.tile([1, 1], F32, tag="zs", name=f"zs{b}")
            nc.scalar.activation(expv[:], ssb[:], AF.Exp,
                                 bias=msh[:, 0:1], accum_out=zs[:, 0:1])
            rz = small_p.tile([1, 1], F32, tag="rz", name=f"rz{b}")
            nc.vector.reciprocal(rz[:], zs[:])
            w_sb = vec_p.tile([1, S], F32, tag="vec", name=f"wsb{b}")
            nc.vector.tensor_scalar_mul(w_sb[:], expv[:], rz[:, 0:1])
            nc.sync.dma_start(w_o[b:b + 1, :], w_sb[:])
            ws_sb = vec_p.tile([1, E], F32, tag="vec", name=f"wssb{b}")
            nc.vector.tensor_scalar_mul(ws_sb[:], acc_sb[:], rz[:, 0:1])
            nc.sync.dma_start(ws_o[b:b + 1, :], ws_sb[:])

        # ---------------- schedule ----------------
        meTl = [meTl_p.tile([128, 512], BF16, tag="meTl", name=f"meTl{et}")
                for et in range(NET)]
        meTh = [meTh_p.tile([128, 512], BF16, tag="meTh", name=f"meTh{et}")
                for et in range(NET)]

        nat00 = load_chunk(0, 0)            # enc b0 chunk0 (gpsimd queue)
        encT00 = transpose_chunk(0, 0, nat00)
        me_group(0, meTl)                   # M_e h-tiles 0-3 (sync queue)
        bias_setup()                        # mh DMAs + DVE bias
        me_group(4, meTh)                   # M_e h-tiles 4-7

        prev = (0, 0, nat00, encT00)
        ssb = {}
        acc = {}

        def get_ssb(b):
            if b not in ssb:
                ssb[b] = vec_p.tile([1, S], F32, tag="vec", name=f"ssb{b}")
            return ssb[b]

        def get_acc(b):
            if b not in acc:
                acc[b] = vec_p.tile([1, E], F32, tag="vec", name=f"accsb{b}")
            return acc[b]

        seq = [(b, sc) for b in range(BPC) for sc in range(NSC)]
        for i, (b, sc) in enumerate(seq):
            pb, psc, pnat, pencT = prev
            sc_ps = mm1_chunk(pb, psc, pencT)
            if i + 1 < len(seq):
                nb, nsc2 = seq[i + 1]
                nnat = load_chunk(nb, nsc2)
                nencT = transpose_chunk(nb, nsc2, nnat)
            expT = chunk_scores(pb, psc, sc_ps, get_ssb(pb))
            weighted_partial(pb, psc, pnat, expT, get_acc(pb))
            if psc == NSC - 1:
                finalize(pb, ssb[pb], acc[pb])
            if i + 1 < len(seq):
                prev = (nb, nsc2, nnat, nencT)

    nc.compile()
    return nc


_NC = None


def _get_nc():
    global _NC
    if _NC is None:
        _NC = _build()
    return _NC


def kernel(encoded, hidden, mask, M_w, M_b, V_w, V_b, _trace=False,
           _tmpdir=None):
    global LAST_EXEC_NS
    encoded = np.ascontiguousarray(np.asarray(encoded, dtype=np.float32))
    hidden = np.asarray(hidden, dtype=np.float32)
    mask_u8 = np.asarray(mask).astype(np.uint8)
    M_w = np.ascontiguousarray(np.asarray(M_w, dtype=np.float32))
    M_b = np.asarray(M_b, dtype=np.float32)
    V_w = np.asarray(V_w, dtype=np.float32)
    # V_b is unused: softmax(s + c) == softmax(s), and masked entries are
    # exactly -1e9 with or without it.

    mbT = np.ascontiguousarray(M_b.reshape(NHT, 128).T)          # [128, 8]
    vT = np.ascontiguousarray(V_w[0].reshape(NHT, 128).T)        # [128, 8]
    hid2 = np.ascontiguousarray(hidden[:, -1, :])                # [B, H]

    nc = _get_nc()
    in_maps = []
    for c in range(N_CORES):
        sl = slice(c * BPC, (c + 1) * BPC)
        in_maps.append({
            "enc": encoded[sl],
            "hid": np.ascontiguousarray(hid2[sl]),
            "mask": np.ascontiguousarray(mask_u8[sl]),
            "mw": M_w,
            "mbT": mbT,
            "vT": vT,
        })

    res = run_bass_kernel_spmd(nc, in_maps, core_ids=list(range(N_CORES)),
                               trace=_trace, tmpdir=_tmpdir)
    LAST_EXEC_NS = res.exec_time_ns

    weights = np.concatenate([r["w_o"] for r in res.results], axis=0)
    weighted = np.concatenate([r["ws_o"] for r in res.results], axis=0)
    return weighted[:, None, :].astype(np.float32), \
        weights[:, None, :].astype(np.float32)
